# revision 5
# baseline (speedup 1.0000x reference)
"""PrefSimMat (EucDis mode) Trainium2 kernel.

sim[i,j] = 1 - dist[i,j] / ||dist[i,:]||_2,  dist = pairwise Euclidean
distance of the rows of p_u [8192, 256] fp32.

Strategy (8 NeuronCores, data-parallel over query rows):
  - Each core computes a [1024, 8192] tile of the output.
  - Gram-matrix identity: sq[i,j] = ni + nj - 2*g[i,j].  Features are
    quantized once to fp8e4 (e4m3) and contracted in NORMAL matmul mode
    (fp8 streams at bf16 speed, but Fast Weight Load stays enabled --
    measured on this part, DoubleRow serializes LDWEIGHTS and streams
    column pairs, netting ~1.8x SLOWER than 3 normal streams).
  - PE floor is 3 streams per output column: 2 feature chunks (K=128)
    plus one K=3 extension chunk carrying the per-column nj term as
    fp8 hi/mid/lo splits (nj - 256 = 16*hi + mid + lo/16, abs err ~4e-3).
    The per-row terms ni + 256 + eps ride in the ScalarE activation bias,
    so no stream is spent on them.
  - fp8 inputs cut the input DMA 2.6x vs bf16 (2.3 MiB/core), which
    removes the baseline's ~13us PE stall waiting for the rhs load (and
    the extended HAM cold-clock window that stall caused).
  - Row norms are computed analytically on the host (O(N*D)) from the
    quantized features, so device and host are numerically consistent:
    rowsum_i = N*(ni+eps) + sum_j nj_eff - 2 * a_i . (sum_j a_j).
  - ScalarE: t = Sqrt(psum * r2_i + r2_i*(ni+256+eps)) (per-partition
    scale/bias APs) = dist_ij/rownorm_i, written as fp16.
  - VectorE: out = t * (-1) + 1 (fp16 -> fp16, packed-2-byte fast mode).
  - Output DMA'd per 2048-column group (512 KiB transfers) so the last
    chunk's drain is ~1.6us instead of 6.3us.

Raw Bass (no TileContext): the walrus build in this container allows at most
one semaphore wait attached per compute instruction, so all cross-engine
dependencies are standalone wait_ge instructions with hand-rolled semaphores.
CoreSim race rule: every semaphore update crossing a waited threshold must be
ordered by its own issuing engine -> one semaphore per input DMA, and each
(stage parity, group) output-DMA slot gets its own semaphore with
issuing-engine self-waits.
"""

import numpy as np
import ml_dtypes

F8 = ml_dtypes.float8_e4m3   # == mybir.dt.float8e4

N = 8192        # rows of p_u == output dim
D = 256         # feature dim
P = 128         # partitions
NCORES = 8
M_PER_CORE = N // NCORES       # 1024 output rows per core
MC = M_PER_CORE // P           # 8 m-chunks of 128 rows
KE = 3          # extension rows (nj hi/mid/lo)
NT = 512        # matmul free-dim tile (one PSUM bank fp32; N=1024 moving
                # operands fail the walrus ISA check despite the doc)
GW = 2048       # ACT/DVE group width = 4 PSUM banks
NG = N // GW    # 4 groups per m-chunk
EPS = 2.0 ** -3 # keeps sqrt argument positive on the diagonal under
                # PSUM/fp8-split rounding (device excursions ~0.01 observed)
CNJ = 256.0     # nj centering constant (absorbed into the ACT bias)

OUT_DT = np.float16

_CACHE = {}


def _build_nc():
    import concourse.bass as bass
    import concourse.mybir as mybir

    f32 = mybir.dt.float32
    f16 = mybir.dt.float16
    f8 = mybir.dt.float8e4
    AF = mybir.ActivationFunctionType
    ALU = mybir.AluOpType

    nc = bass.Bass()
    l0_d = nc.dram_tensor("l0", [P, M_PER_CORE], f8, kind="ExternalInput")
    l1_d = nc.dram_tensor("l1", [P, M_PER_CORE], f8, kind="ExternalInput")
    r0_d = nc.dram_tensor("r0", [P, N], f8, kind="ExternalInput")
    r1_d = nc.dram_tensor("r1", [P, N], f8, kind="ExternalInput")
    extw_d = nc.dram_tensor("extw", [KE, P], f8, kind="ExternalInput")
    extr_d = nc.dram_tensor("extr", [KE, N], f8, kind="ExternalInput")
    sc_d = nc.dram_tensor("sc", [P, 2 * MC], f32, kind="ExternalInput")
    out_d = nc.dram_tensor("out", [M_PER_CORE, N], f16, kind="ExternalOutput")

    NGI = MC * NG  # 32 pipeline groups

    from contextlib import ExitStack

    with ExitStack() as ctx:
        r0_s = ctx.enter_context(nc.sbuf_tensor("r0_s", [P, N], f8))
        r1_s = ctx.enter_context(nc.sbuf_tensor("r1_s", [P, N], f8))
        l0_s = ctx.enter_context(nc.sbuf_tensor("l0_s", [P, M_PER_CORE], f8))
        l1_s = ctx.enter_context(nc.sbuf_tensor("l1_s", [P, M_PER_CORE], f8))
        extw_s = ctx.enter_context(nc.sbuf_tensor("extw_s", [KE, P], f8))
        extr_s = ctx.enter_context(nc.sbuf_tensor("extr_s", [KE, N], f8))
        sc_s = ctx.enter_context(nc.sbuf_tensor("sc_s", [P, 2 * MC], f32))
        tbuf = ctx.enter_context(nc.sbuf_tensor("tbuf", [P, 4 * GW], f16))
        stage = ctx.enter_context(nc.sbuf_tensor("stage", [P, 2 * N], f16))
        ps = ctx.enter_context(nc.psum_tensor("ps", [P, 2 * GW], f32))
        rhs_g_sems = [
            [ctx.enter_context(nc.semaphore(f"in_r{c}_{g}")) for c in range(2)]
            for g in range(NG)
        ]
        in_l = ctx.enter_context(nc.semaphore("in_l"))
        in_ext = ctx.enter_context(nc.semaphore("in_ext"))
        in_sc = ctx.enter_context(nc.semaphore("in_sc"))
        sem_mm = ctx.enter_context(nc.semaphore("sem_mm"))
        sem_act = ctx.enter_context(nc.semaphore("sem_act"))
        sem_ts = ctx.enter_context(nc.semaphore("sem_ts"))
        out_sems = [
            [ctx.enter_context(nc.semaphore(f"dma_o{par}_{g}")) for g in range(NG)]
            for par in range(2)
        ]
        block = ctx.enter_context(nc.Block())
        r_sb = [r0_s, r1_s]
        r_dr = [r0_d, r1_d]

        @block.sync
        def _(sync):
            sync.dma_start(sc_s[:, :], sc_d[:, :]).then_inc(in_sc, 16)
            sync.dma_start(l0_s[:, :], l0_d[:, :]).then_inc(in_l, 16)
            sync.dma_start(l1_s[:, :], l1_d[:, :]).then_inc(in_l, 16)
            sync.dma_start(extw_s[:, :], extw_d[:, :]).then_inc(in_ext, 16)
            sync.dma_start(extr_s[:, :], extr_d[:, :]).then_inc(in_ext, 16)
            for g in range(NG):
                c0, c1 = g * GW, (g + 1) * GW
                for c in range(2):
                    sync.dma_start(
                        r_sb[c][:, c0:c1], r_dr[c][:, c0:c1]
                    ).then_inc(rhs_g_sems[g][c], 16)
            for m in range(MC):
                for g in range(NG):
                    sync.wait_ge(sem_ts, m * NG + g + 1)
                    if m >= 2:
                        # self-serialize this parity/group DMA slot
                        sync.wait_ge(out_sems[m % 2][g], 16 * (m // 2))
                    sync.dma_start(
                        out_d[m * P : (m + 1) * P, g * GW : (g + 1) * GW],
                        stage[:, (m % 2) * N + g * GW : (m % 2) * N + (g + 1) * GW],
                    ).then_inc(out_sems[m % 2][g], 16)

        @block.tensor
        def _(tensor):
            tensor.wait_ge(in_l, 32)
            tensor.wait_ge(in_ext, 32)
            for m in range(MC):
                lsl0 = l0_s[:, m * P : (m + 1) * P]
                lsl1 = l1_s[:, m * P : (m + 1) * P]
                for g in range(NG):
                    gi = m * NG + g
                    if m == 0:
                        for s in rhs_g_sems[g]:
                            tensor.wait_ge(s, 16)
                    if gi >= 2:
                        tensor.wait_ge(sem_act, gi - 1)
                    inst = None
                    for j in range(GW // NT):
                        n0 = g * GW + j * NT
                        p0 = (gi % 2) * GW + j * NT
                        tensor.matmul(
                            ps[:, p0 : p0 + NT],
                            lsl0,
                            r0_s[:, n0 : n0 + NT],
                            start=True,
                            stop=False,
                        )
                        tensor.matmul(
                            ps[:, p0 : p0 + NT],
                            lsl1,
                            r1_s[:, n0 : n0 + NT],
                            start=False,
                            stop=False,
                        )
                        inst = tensor.matmul(
                            ps[:, p0 : p0 + NT],
                            extw_s[:, :],
                            extr_s[:, n0 : n0 + NT],
                            start=False,
                            stop=True,
                        )
                    inst.then_inc(sem_mm, 1)

        @block.scalar
        def _(scalar):
            scalar.wait_ge(in_sc, 16)
            for gi in range(NGI):
                m = gi // NG
                scalar.wait_ge(sem_mm, gi + 1)
                if gi >= 4:
                    scalar.wait_ge(sem_ts, gi - 3)
                scalar.activation(
                    tbuf[:, (gi % 4) * GW : (gi % 4 + 1) * GW],
                    ps[:, (gi % 2) * GW : (gi % 2 + 1) * GW],
                    AF.Sqrt,
                    scale=sc_s[:, m : m + 1],
                    bias=sc_s[:, MC + m : MC + m + 1],
                ).then_inc(sem_act, 1)

        @block.vector
        def _(vector):
            for gi in range(NGI):
                m, g = divmod(gi, NG)
                vector.wait_ge(sem_act, gi + 1)
                if m >= 2:
                    vector.wait_ge(out_sems[m % 2][g], 16 * (m // 2))
                vector.tensor_scalar(
                    stage[:, (m % 2) * N + g * GW : (m % 2) * N + (g + 1) * GW],
                    tbuf[:, (gi % 4) * GW : (gi % 4 + 1) * GW],
                    -1.0,
                    1.0,
                    op0=ALU.mult,
                    op1=ALU.add,
                ).then_inc(sem_ts, 1)

    return nc


def _prep_inputs(p_u):
    """Host-side O(N*D) prep: fp8 cast/transpose, norms, row sums."""
    a8 = p_u.astype(F8)                       # quantize features once
    af = a8.astype(np.float32)
    a64 = af.astype(np.float64)
    ni64 = np.einsum("ij,ij->i", a64, a64)    # [N] norms of quantized rows

    # nj extension rows: nj - CNJ = 16*hi + mid + lo/16 (fp8e4 splits)
    njp = ni64 - CNJ
    hi8 = (njp / 16.0).astype(np.float32).astype(F8)
    hi = hi8.astype(np.float64)
    r = njp - 16.0 * hi
    mid8 = r.astype(np.float32).astype(F8)
    mid = mid8.astype(np.float64)
    lo8 = (16.0 * (r - mid)).astype(np.float32).astype(F8)
    lo = lo8.astype(np.float64)
    nj_eff = CNJ + 16.0 * hi + mid + lo / 16.0

    t64 = a64.sum(axis=0)                     # [D]
    rowsum = N * ni64 + nj_eff.sum() - 2.0 * (a64 @ t64) + N * EPS
    r2 = 1.0 / rowsum                         # [N] f64
    bias64 = r2 * (ni64 + CNJ + EPS)

    aT8 = np.ascontiguousarray(a8.T)          # [256, 8192] fp8
    r0 = aT8[0:P]
    r1 = aT8[P : 2 * P]
    extr = np.stack([hi8, mid8, lo8], axis=0) # [3, 8192] fp8
    extw = np.zeros((KE, P), dtype=F8)
    extw[0, :] = F8(16.0)
    extw[1, :] = F8(1.0)
    extw[2, :] = F8(1.0 / 16.0)

    m2 = (-2.0 * af).astype(F8)               # exact fp8 doubling
    m2T = np.ascontiguousarray(m2.T)          # [256, 8192]
    r2f = r2.astype(np.float32)
    biasf = bias64.astype(np.float32)

    in_maps = []
    for c in range(NCORES):
        sl = slice(c * M_PER_CORE, (c + 1) * M_PER_CORE)
        l0 = np.ascontiguousarray(m2T[0:P, sl])
        l1 = np.ascontiguousarray(m2T[P : 2 * P, sl])
        sc = np.concatenate(
            [
                np.ascontiguousarray(r2f[sl].reshape(MC, P).T),
                np.ascontiguousarray(biasf[sl].reshape(MC, P).T),
            ],
            axis=1,
        ).astype(np.float32)                  # [128, 16]
        in_maps.append(
            {"l0": l0, "l1": l1, "r0": r0, "r1": r1,
             "extw": extw, "extr": extr, "sc": sc}
        )
    return in_maps


def kernel(p_u):
    from concourse.bass_utils import run_bass_kernel_spmd

    p_u = np.asarray(p_u, dtype=np.float32)
    assert p_u.shape == (N, D)

    if "nc" not in _CACHE:
        _CACHE["nc"] = _build_nc()
    nc = _CACHE["nc"]

    in_maps = _prep_inputs(p_u)
    trace = bool(_CACHE.get("trace"))
    res = run_bass_kernel_spmd(nc, in_maps, core_ids=list(range(NCORES)), trace=trace)
    _CACHE["last_result"] = res
    out = np.concatenate(
        [res.results[c]["out"].astype(np.float32) for c in range(NCORES)], axis=0
    )
    return out
